# revision 14
# baseline (speedup 1.0000x reference)
"""Trainium2 Bass kernel for nn_BlockRasterizer.

8 NeuronCores, SPMD.  Host does selection/top-K/wv + per-pair weight
gather + load balancing into fixed pair slots; all data-dependent
structure is per-core input DATA.  Device does the full per-point MLP
(fp32r matmuls), blending via selector-matmul reductions, and the
sequential compositing via DVE prefix scans.

No column tile_position is used (broken on this stack); small matmuls
are packed via row-tiling, block-diagonal K-merge of slot pairs, and
M-shifted accumulation (sigma densification into a shared PSUM bank).
"""
import sys
for p in ('/opt/trn_rl_repo', '/opt/trn_rl_repo/concourse'):
    if p not in sys.path:
        sys.path.insert(0, p)

from contextlib import ExitStack
import numpy as np

import concourse.bass as bass
import concourse.bacc as bacc
import concourse.tile as tile
from concourse import mybir
from concourse.bass_utils import run_bass_kernel_spmd

F32 = mybir.dt.float32
F32R = mybir.dt.float32r
AF = mybir.ActivationFunctionType
ALU = mybir.AluOpType

R, NB, K, S, H, FEAT, EMB, NAPP = 256, 64, 8, 256, 128, 32, 16, 100
STEP, VIS_T, TERM_T, T_EPS = 0.5, 0.01, 0.99, 1e-4
N_CORES = 8
RC = R // N_CORES

# blobP layout (per quad, [4, P_W]): slot c: pos at 256c (1024), w1 at 1024+128c
P_W = 1536
# blobA layout (per quad, [128, A_W]):
A_W2 = 0             # slot c at cols +128c (512)
A_L3 = 512           # per pair j, slot i: lhsT [128,66] (264 total)
A_SIG = 776          # per pair j: sigma densify lhsT [66,64] (128)
A_WC1 = 904          # per pair j: blockdiag [64,128] (256)
A_WC2 = 1160         # per pair j: blockdiag [128,8] (16)
A_W = 1176
# blobB layout (per quad, [8, B_W]): per pair j: selC [8,96] | cwv [8,256]
B_W = 704


def _build_nc(NS: int) -> bass.Bass:
    NQ = NS // 4
    NP = NS // 2                 # slot pairs
    NG = (NP + 31) // 32         # sigma-dense groups of 32 pairs (64 slots)
    nc = bacc.Bacc("TRN2", target_bir_lowering=False, debug=False)

    def din(name, shape, dt=F32):
        return nc.dram_tensor(name, list(shape), dt, kind="ExternalInput").ap()

    blobA_d = din("blobA", (NQ, 128, A_W), F32R)
    blobP_d = din("blobP", (NQ, 4, P_W), F32R)
    blobB_d = din("blobB", (NQ, 8, B_W), F32R)
    b2g_d = din("b2g", (H, NS))
    bc1g_d = din("bc1g", (H, NP))
    bc2g_d = din("bc2g", (8, NP))
    wvd_d = din("wvd", (64, NG * S))
    bdd_d = din("bdd", (64, NG))
    selD_d = din("selD", (64, NG * 32), F32R)
    tw_d = din("tw", (RC, S))
    tmask_d = din("tmask", (RC, S))
    tg_d = din("tg", (RC, S))

    rgb_o = nc.dram_tensor("rgb_o", [RC, 3], F32, kind="ExternalOutput").ap()
    acc_o = nc.dram_tensor("acc_o", [RC, 1], F32, kind="ExternalOutput").ap()
    dep_o = nc.dram_tensor("dep_o", [RC, 1], F32, kind="ExternalOutput").ap()

    with tile.TileContext(nc) as tc, ExitStack() as ctx:
        res = ctx.enter_context(tc.tile_pool(name="res", bufs=1))
        sbuf = ctx.enter_context(tc.tile_pool(name="sbuf", bufs=1))
        psum = ctx.enter_context(tc.tile_pool(name="psum", bufs=1, space="PSUM"))

        # ---- resident inputs ----
        b2g = res.tile([H, NS], F32)
        nc.sync.dma_start(b2g[:], b2g_d[:])
        bc1g = res.tile([H, NP], F32)
        nc.sync.dma_start(bc1g[:], bc1g_d[:])
        bc2g = res.tile([8, NP], F32)
        nc.sync.dma_start(bc2g[:], bc2g_d[:])
        wvd = res.tile([64, NG * S], F32)
        nc.sync.dma_start(wvd[:], wvd_d[:])
        bdd = res.tile([64, NG], F32)
        nc.sync.dma_start(bdd[:], bdd_d[:])
        selDt = res.tile([64, NG * 32], F32R)
        nc.sync.dma_start(selDt[:], selD_d[:])
        tw_t = res.tile([RC, S], F32)
        nc.sync.dma_start(tw_t[:], tw_d[:])
        tmask_t = res.tile([RC, S], F32)
        nc.sync.dma_start(tmask_t[:], tmask_d[:])
        tg_t = res.tile([RC, S], F32)
        nc.sync.dma_start(tg_t[:], tg_d[:])
        ones_t = res.tile([RC, S], F32)
        nc.gpsimd.memset(ones_t[:], 1.0)
        zeros_t = res.tile([RC, S], F32)
        nc.gpsimd.memset(zeros_t[:], 0.0)

        # persistent PSUM accumulators
        colacc = psum.tile([96, S], F32)
        sigd = psum.tile([64, NG * S], F32)
        densps = psum.tile([RC, S], F32)

        # ---- main loop over quads (2 pairs each) ----
        for q in range(NQ):
            bA = sbuf.tile([128, A_W], F32R, name=f"bA_{q}", tag="bA", bufs=3)
            nc.sync.dma_start(bA[:], blobA_d[q])
            bP = sbuf.tile([4, P_W], F32R, name=f"bP_{q}", tag="bP", bufs=3)
            nc.gpsimd.dma_start(bP[:], blobP_d[q])
            bB = sbuf.tile([8, B_W], F32R, name=f"bB_{q}", tag="bB", bufs=3)
            nc.gpsimd.dma_start(bB[:], blobB_d[q])

            fs2 = None
            fsps = psum.tile([66, 2 * S], F32, name=f"fsps_{q}",
                             tag="fsps", bufs=1)
            for j in range(2):
                p_ = 2 * q + j
                c0, c1 = 2 * j, 2 * j + 1

                l1ps = psum.tile([128, 2 * S], F32, name=f"l1ps_{p_}",
                                 tag="l1ps", bufs=1)
                for i, c in enumerate((c0, c1)):
                    nc.tensor.matmul(l1ps[:, S*i:S*i+S],
                                     bP[0:4, 1024+H*c:1024+H*c+H],
                                     bP[0:4, S*c:S*c+S],
                                     start=True, stop=True)
                h1p = sbuf.tile([128, 2 * S], F32R, name=f"h1p_{p_}",
                                tag="h1p", bufs=2)
                if p_ % 2 == 0:
                    nc.scalar.activation(h1p[:], l1ps[:], AF.Relu)
                else:
                    nc.vector.tensor_scalar(h1p[:], l1ps[:], 0.0, None, ALU.max)

                l2ps = psum.tile([128, 2 * S], F32, name=f"l2ps_{p_}",
                                 tag="l2ps", bufs=1)
                h2x = []
                for i, c in enumerate((c0, c1)):
                    s_ = 4 * q + c
                    nc.tensor.matmul(l2ps[:, S*i:S*i+S],
                                     bA[:, A_W2+H*c:A_W2+H*c+H],
                                     h1p[:, S*i:S*i+S],
                                     start=True, stop=True)
                    h2 = sbuf.tile([H, S], F32R, name=f"h2_{p_}_{i}",
                                   tag=f"h2_{i}", bufs=2)
                    bias = b2g[:, s_:s_+1]
                    if s_ % 2 == 0:
                        nc.scalar.activation(h2[:], l2ps[:, S*i:S*i+S],
                                             AF.Relu, bias=bias)
                    else:
                        nc.vector.tensor_scalar(h2[:], l2ps[:, S*i:S*i+S],
                                                bias, 0.0, ALU.add, ALU.max)
                    h2x.append(h2)

                # L3 accumulate-merge: rows 0-31 featA, 32-63 featB, 64/65 sig
                for i in range(2):
                    base = A_L3 + 132*j + 66*i
                    nc.tensor.matmul(fsps[:, S*j:S*j+S],
                                     bA[:, base:base+66],
                                     h2x[i][:],
                                     start=(i == 0), stop=(i == 1))
            fs2 = sbuf.tile([66, 2 * S], F32R, name=f"fs2_{q}",
                            tag="fs2", bufs=2)
            if q % 2 == 0:
                nc.scalar.activation(fs2[:], fsps[:], AF.Copy)
            else:
                nc.vector.tensor_copy(fs2[:], fsps[:])

            for j in range(2):
                p_ = 2 * q + j
                g_, jg = divmod(p_, 32)
                fs2v = fs2[:, S*j:S*j+S]

                nc.tensor.matmul(sigd[:, S*g_:S*g_+S],
                                 bA[0:66, A_SIG+64*j:A_SIG+64*j+64],
                                 fs2v,
                                 start=(jg == 0),
                                 stop=(jg == 31 or p_ == NP - 1))
                l4c = psum.tile([128, 2 * S], F32, name=f"l4c_{p_}",
                                tag="l4c", bufs=1)
                nc.tensor.matmul(l4c[:, 0:S],
                                 bA[0:64, A_WC1+128*j:A_WC1+128*j+128],
                                 fs2[0:64, S*j:S*j+S],
                                 start=True, stop=True)
                hcx = sbuf.tile([128, S], F32R, name=f"hcx_{p_}",
                                tag="hcx", bufs=2)
                bias = bc1g[:, p_:p_+1]
                if p_ % 2 == 0:
                    nc.scalar.activation(hcx[:], l4c[:, 0:S], AF.Relu, bias=bias)
                else:
                    nc.vector.tensor_scalar(hcx[:], l4c[:, 0:S], bias, 0.0,
                                            ALU.add, ALU.max)

                nc.tensor.matmul(l4c[0:8, S:2*S],
                                 bA[:, A_WC2+8*j:A_WC2+8*j+8],
                                 hcx[:],
                                 start=True, stop=True)
                colsb = sbuf.tile([8, S], F32, name=f"colsb_{p_}",
                                  tag="colsb", bufs=2)
                nc.scalar.activation(colsb[:], l4c[0:8, S:2*S], AF.Tanh,
                                     bias=bc2g[:, p_:p_+1], scale=0.5)
                cw = sbuf.tile([8, S], F32R, name=f"cw_{p_}", tag="cw", bufs=2)
                nc.vector.scalar_tensor_tensor(
                    cw[:], colsb[:], 1.0, bB[:, 352*j+96:352*j+96+S],
                    ALU.add, ALU.mult)
                nc.tensor.matmul(colacc[:], bB[:, 352*j:352*j+96],
                                 cw[:],
                                 start=(p_ == 0), stop=(p_ == NP - 1))

        # ---- sigma tail: softplus(z) = ln(exp(min(z+bd,40)) + 1) ----
        sigs = res.tile([64, NG * S], F32)
        for g_ in range(NG):
            nc.vector.tensor_scalar(sigs[:, S*g_:S*g_+S], sigd[:, S*g_:S*g_+S],
                                    bdd[:, g_:g_+1], 40.0, ALU.add, ALU.min)
        sige = res.tile([64, NG * S], F32)
        nc.scalar.activation(sige[:], sigs[:], AF.Exp)
        sigl = res.tile([64, NG * S], F32)
        nc.scalar.activation(sigl[:], sige[:], AF.Ln, bias=1.0)
        sigv = res.tile([64, NG * S], F32R)
        nc.vector.tensor_tensor(sigv[:], sigl[:], wvd[:], ALU.mult)
        for g_ in range(NG):
            nc.tensor.matmul(densps[:], selDt[:, 32*g_:32*g_+32],
                             sigv[:, S*g_:S*g_+S],
                             start=(g_ == 0), stop=(g_ == NG - 1))

        # ---- compositing ----
        cp = res
        e_t = cp.tile([RC, S], F32)
        nc.scalar.activation(e_t[:], densps[:], AF.Exp, scale=-0.5)
        a_t = cp.tile([RC, S], F32)
        nc.vector.tensor_scalar(a_t[:], e_t[:], -1.0, 1.0, ALU.mult, ALU.add)
        Ti = cp.tile([RC, S], F32)
        nc.vector.tensor_tensor_scan(Ti[:], e_t[:], ones_t[:], 1.0,
                                     ALU.mult, ALU.mult)
        Tu = cp.tile([RC, S], F32)
        nc.gpsimd.memset(Tu[:, 0:1], 1.0)
        nc.vector.tensor_copy(Tu[:, 1:S], Ti[:, 0:S-1])
        wu = cp.tile([RC, S], F32)
        nc.vector.tensor_tensor(wu[:], Tu[:], a_t[:], ALU.mult)
        Ci = cp.tile([RC, S], F32)
        nc.vector.tensor_tensor_scan(Ci[:], wu[:], zeros_t[:], 0.0,
                                     ALU.add, ALU.add)
        Au = cp.tile([RC, S], F32)
        nc.gpsimd.memset(Au[:, 0:1], 0.0)
        nc.vector.tensor_copy(Au[:, 1:S], Ci[:, 0:S-1])
        m1 = cp.tile([RC, S], F32)
        nc.vector.tensor_scalar(m1[:], Tu[:], T_EPS, None, ALU.is_gt)
        m2 = cp.tile([RC, S], F32)
        nc.vector.tensor_scalar(m2[:], Au[:], TERM_T, None, ALU.is_le)
        wgt = cp.tile([RC, S], F32)
        nc.vector.tensor_tensor(wgt[:], wu[:], m1[:], ALU.mult)
        nc.vector.tensor_tensor(wgt[:], wgt[:], m2[:], ALU.mult)
        nc.vector.tensor_tensor(wgt[:], wgt[:], tmask_t[:], ALU.mult)
        twc = cp.tile([RC, S], F32)
        nc.vector.tensor_scalar(twc[:], tw_t[:], 1e-12, None, ALU.max)
        rcp = cp.tile([RC, S], F32)
        nc.vector.reciprocal(rcp[:], twc[:])
        rgb3 = cp.tile([RC, 3], F32)
        for ch in range(3):
            cn = cp.tile([RC, S], F32, name=f"cn_{ch}", tag="cn", bufs=2)
            nc.vector.tensor_tensor(cn[:], colacc[32*ch:32*ch+RC, :], rcp[:],
                                    ALU.mult)
            wc_ = cp.tile([RC, S], F32, name=f"wc_{ch}", tag="wc", bufs=2)
            nc.vector.tensor_tensor(wc_[:], cn[:], wgt[:], ALU.mult)
            nc.vector.tensor_reduce(rgb3[:, ch:ch+1], wc_[:],
                                    mybir.AxisListType.X, ALU.add)
        acc_t = cp.tile([RC, 1], F32)
        nc.vector.tensor_reduce(acc_t[:], wgt[:], mybir.AxisListType.X, ALU.add)
        wt_t = cp.tile([RC, S], F32)
        nc.vector.tensor_tensor(wt_t[:], wgt[:], tg_t[:], ALU.mult)
        dep_t = cp.tile([RC, 1], F32)
        nc.vector.tensor_reduce(dep_t[:], wt_t[:], mybir.AxisListType.X, ALU.add)
        nc.sync.dma_start(rgb_o[:], rgb3[:])
        nc.sync.dma_start(acc_o[:], acc_t[:])
        nc.sync.dma_start(dep_o[:], dep_t[:])

    nc.compile()
    return nc


_NC_CACHE: dict = {}


def _get_nc(NS: int) -> bass.Bass:
    if NS not in _NC_CACHE:
        _NC_CACHE[NS] = _build_nc(NS)
    return _NC_CACHE[NS]


def _host_prep(ray_origins, ray_directions, block_centers, block_radii,
               appearance_ids, exposure_values, near, far,
               W1, b1, W2, b2, Wd, bd, Wf, Wc1, bc1, Wc2, bc2, app_emb):
    f = np.float32
    o = np.asarray(ray_origins, f); d = np.asarray(ray_directions, f)
    bc_ = np.asarray(block_centers, f); br = np.asarray(block_radii, f)
    aid = np.asarray(appearance_ids).astype(np.int64)
    expo = np.asarray(exposure_values, f)
    near = np.asarray(near, f); far = np.asarray(far, f)

    oc = o[:, None, :] - bc_[None]
    a = np.sum(d * d, -1)[:, None]
    bq = f(2.0) * np.sum(oc * d[:, None, :], -1)
    cq = np.sum(oc * oc, -1) - br[None] ** 2
    disc = bq * bq - f(4.0) * a * cq
    sq = np.sqrt(np.where(disc > 0, disc, f(1.0)), dtype=f)
    sq = np.where(disc >= 0, sq, f(0.0))
    t1 = (-bq - sq) / (f(2.0) * a)
    t2 = (-bq + sq) / (f(2.0) * a)
    thit = np.where(t1 > 0, t1, t2)
    valid = (disc >= 0) & (thit > 0)
    hit = o[:, None, :] + thit[..., None] * d[:, None, :]
    dist = np.sqrt(np.sum((hit - bc_[None]) ** 2, -1), dtype=f)
    dist = np.where(valid, dist, f(np.inf))
    sel_idx = np.argsort(dist, axis=1, kind='stable')[:, :K]
    seld = np.take_along_axis(dist, sel_idx, 1)
    sel_valid = np.isfinite(seld)

    t_grid = near[:, None] + f(STEP) * np.arange(S, dtype=f)[None]
    pos = o[:, None, :] + t_grid[..., None] * d[:, None, :]
    csel = bc_[sel_idx]
    dpb = np.sqrt(np.sum((pos[:, :, None, :] - csel[:, None, :, :]) ** 2, -1),
                  dtype=f)
    inv = np.where(sel_valid[:, None, :], f(1.0) / (dpb + f(1e-6)), f(0.0))
    wsum = inv.sum(-1, keepdims=True, dtype=f)
    w = np.where(wsum > 0, inv / np.maximum(wsum, f(1e-12)), f(0.0)).astype(f)
    vis = (w >= f(VIS_T)) & sel_valid[:, None, :]
    wv = np.where(vis, w, f(0.0)).astype(f)
    tw = wv.sum(-1, dtype=f)

    W1 = np.asarray(W1, f); b1 = np.asarray(b1, f); W2 = np.asarray(W2, f)
    b2 = np.asarray(b2, f); Wd = np.asarray(Wd, f); bd = np.asarray(bd, f)
    Wf = np.asarray(Wf, f); Wc1 = np.asarray(Wc1, f); bc1 = np.asarray(bc1, f)
    Wc2 = np.asarray(Wc2, f); bc2 = np.asarray(bc2, f)
    app_emb = np.asarray(app_emb, f)

    emb_rk = app_emb[sel_idx, aid[:, None]]
    cin_const = np.concatenate([
        np.broadcast_to(d[:, None, :], (R, K, 3)),
        emb_rk,
        np.broadcast_to(expo[:, None, :], (R, K, 1)),
    ], -1).astype(f)
    Wc1b = Wc1[:, FEAT:, :]
    bc1p = bc1[sel_idx] + np.einsum('rkc,rkcd->rkd', cin_const,
                                    Wc1b[sel_idx]).astype(f)

    nr = sel_valid.sum(1)
    order = np.argsort(-nr, kind='stable')
    core_rays = [[] for _ in range(N_CORES)]
    for i, r_ in enumerate(order):
        rnd = i // N_CORES
        c_ = i % N_CORES if rnd % 2 == 0 else N_CORES - 1 - (i % N_CORES)
        core_rays[c_].append(int(r_))
    loads = [int(nr[cr].sum()) for cr in core_rays]
    NS = max(8, ((max(loads) + 7) // 8) * 8)
    NQ = NS // 4
    NP = NS // 2
    NG = (NP + 31) // 32

    in_maps = []
    for core in range(N_CORES):
        rays = core_rays[core]
        slots = []
        for rl, rg in enumerate(rays):
            for k_ in range(K):
                if sel_valid[rg, k_]:
                    slots.append((rl, rg, k_, int(sel_idx[rg, k_])))
        slots += [None] * (NS - len(slots))

        blobA = np.zeros((NQ, 128, A_W), f)
        blobP = np.zeros((NQ, 4, P_W), f)
        blobB = np.zeros((NQ, 8, B_W), f)
        b2g_a = np.zeros((H, NS), f)
        bc1g_a = np.zeros((H, NP), f)
        bc2g_a = np.zeros((8, NP), f)
        wvd_a = np.zeros((64, NG * S), f)
        bdd_a = np.zeros((64, NG), f)
        selD_a = np.zeros((64, NG * 32), f)

        for s_ in range(NS):
            sl = slots[s_]
            if sl is None:
                continue
            rl, rg, k_, b_ = sl
            q, c = divmod(s_, 4)
            j, i = divmod(c, 2)
            p_ = 2 * q + j
            g_, jg = divmod(p_, 32)
            blobP[q, 0:3, S*c:S*c+S] = pos[rg].T
            blobP[q, 3, S*c:S*c+S] = 1.0
            blobP[q, 0:3, 1024+H*c:1024+H*c+H] = W1[b_]
            blobP[q, 3, 1024+H*c:1024+H*c+H] = b1[b_]
            blobA[q, :, A_W2+H*c:A_W2+H*c+H] = W2[b_]
            base = A_L3 + 132*j + 66*i
            blobA[q, :, base+32*i:base+32*i+32] = Wf[b_]
            blobA[q, :, base+64+i] = Wd[b_, :, 0]
            blobA[q, 64+i, A_SIG+64*j + 2*jg+i] = 1.0
            blobA[q, 32*i:32*i+FEAT,
                  A_WC1+128*j+64*i:A_WC1+128*j+64*i+64] = Wc1[b_][:FEAT]
            blobA[q, 64*i:64*i+64, A_WC2+8*j+4*i:A_WC2+8*j+4*i+3] = Wc2[b_]
            b2g_a[:, s_] = b2[b_]
            bc1g_a[64*i:64*i+64, p_] = bc1p[rg, k_]
            bc2g_a[4*i:4*i+3, p_] = f(0.5) * bc2[b_]
            for ch in range(3):
                blobB[q, 4*i+ch, 352*j + 32*ch + rl] = 0.5
                blobB[q, 4*i+ch, 352*j + 96:352*j + 96 + S] = wv[rg, :, k_]
            wvd_a[2*jg+i, S*g_:S*g_+S] = wv[rg, :, k_]
            bdd_a[2*jg+i, g_] = bd[b_, 0]
            selD_a[2*jg+i, 32*g_ + rl] = 1.0

        in_maps.append(dict(
            blobA=blobA, blobP=blobP, blobB=blobB, b2g=b2g_a, bc1g=bc1g_a, bc2g=bc2g_a,
            wvd=wvd_a, bdd=bdd_a, selD=selD_a,
            tw=np.ascontiguousarray(tw[rays]),
            tmask=(t_grid[rays] < far[rays, None]).astype(f),
            tg=np.ascontiguousarray(t_grid[rays]),
        ))

    return NS, core_rays, in_maps


def kernel(**inputs):
    NS, core_rays, in_maps = _host_prep(**inputs)
    nc = _get_nc(NS)
    res = run_bass_kernel_spmd(nc, in_maps, list(range(N_CORES)))
    rgb = np.zeros((R, 3), np.float32)
    depth = np.zeros((R,), np.float32)
    acc = np.zeros((R,), np.float32)
    for core in range(N_CORES):
        r = res.results[core]
        rays = core_rays[core]
        rgb[rays] = r["rgb_o"]
        a = r["acc_o"][:, 0]
        acc[rays] = a
        depth[rays] = r["dep_o"][:, 0] / np.maximum(a, np.float32(1e-6))
    return rgb, depth, acc


# revision 17
# speedup vs baseline: 1.0665x; 1.0665x over previous
"""Trainium2 Bass kernel for nn_BlockRasterizer.

8 NeuronCores, SPMD.  Host does selection/top-K/wv + per-pair weight
gather + load balancing into fixed pair slots; all data-dependent
structure is per-core input DATA.  Device does the full per-point MLP
(fp32r matmuls), blending via selector-matmul reductions, and the
sequential compositing via DVE prefix scans.

No column tile_position is used (broken on this stack); small matmuls
are packed via row-tiling, block-diagonal K-merge of slot pairs, and
M-shifted accumulation (sigma densification into a shared PSUM bank).
"""
import sys
for p in ('/opt/trn_rl_repo', '/opt/trn_rl_repo/concourse'):
    if p not in sys.path:
        sys.path.insert(0, p)

from contextlib import ExitStack
import numpy as np

import concourse.bass as bass
import concourse.bacc as bacc
import concourse.tile as tile
from concourse import mybir
from concourse.bass_utils import run_bass_kernel_spmd

F32 = mybir.dt.float32
F32R = mybir.dt.float32r
AF = mybir.ActivationFunctionType
ALU = mybir.AluOpType

R, NB, K, S, H, FEAT, EMB, NAPP = 256, 64, 8, 256, 128, 32, 16, 100
STEP, VIS_T, TERM_T, T_EPS = 0.5, 0.01, 0.99, 1e-4
N_CORES = 8
RC = R // N_CORES

# blobP layout (per quad, [4, P_W]): slot c: pos at 256c (1024), w1 at 1024+128c
P_W = 1536
# blobA layout (per quad, [128, A_W]):
A_W2 = 0             # slot c at cols +128c (512)
A_L3 = 512           # per pair j, slot i: lhsT [128,66] (264 total)
A_SIG = 776          # per pair j: sigma densify lhsT [66,64] (128)
A_WC1 = 904          # per pair j: blockdiag [64,128] (256)
A_WC2 = 1160         # per pair j: blockdiag [128,8] (16)
A_W = 1176
# blobB layout (per quad, [8, B_W]): per pair j: selC [8,96] | cwv [8,256]
B_W = 704


def _build_nc(NS: int) -> bass.Bass:
    NQ = NS // 4
    NP = NS // 2                 # slot pairs
    NG = (NP + 31) // 32         # sigma-dense groups of 32 pairs (64 slots)
    nc = bacc.Bacc("TRN2", target_bir_lowering=False, debug=False)

    def din(name, shape, dt=F32):
        return nc.dram_tensor(name, list(shape), dt, kind="ExternalInput").ap()

    blobA_d = din("blobA", (NQ, 128, A_W), F32R)
    blobP_d = din("blobP", (NQ, 4, P_W), F32R)
    blobB_d = din("blobB", (NQ, 8, B_W), F32R)
    b2g_d = din("b2g", (H, NS))
    bc1g_d = din("bc1g", (H, NP))
    bc2g_d = din("bc2g", (8, NP))
    wvd_d = din("wvd", (64, NG * S))
    bdd_d = din("bdd", (64, NG))
    selD_d = din("selD", (64, NG * 32), F32R)
    tw_d = din("tw", (RC, S))
    tmask_d = din("tmask", (RC, S))
    tg_d = din("tg", (RC, S))

    rgb_o = nc.dram_tensor("rgb_o", [RC, 3], F32, kind="ExternalOutput").ap()
    acc_o = nc.dram_tensor("acc_o", [RC, 1], F32, kind="ExternalOutput").ap()
    dep_o = nc.dram_tensor("dep_o", [RC, 1], F32, kind="ExternalOutput").ap()

    with tile.TileContext(nc) as tc, ExitStack() as ctx:
        res = ctx.enter_context(tc.tile_pool(name="res", bufs=1))
        sbuf = ctx.enter_context(tc.tile_pool(name="sbuf", bufs=1))
        psum = ctx.enter_context(tc.tile_pool(name="psum", bufs=1, space="PSUM"))

        # ---- resident inputs ----
        b2g = res.tile([H, NS], F32)
        nc.sync.dma_start(b2g[:], b2g_d[:])
        bc1g = res.tile([H, NP], F32)
        nc.sync.dma_start(bc1g[:], bc1g_d[:])
        bc2g = res.tile([8, NP], F32)
        nc.sync.dma_start(bc2g[:], bc2g_d[:])
        wvd = res.tile([64, NG * S], F32)
        nc.sync.dma_start(wvd[:], wvd_d[:])
        bdd = res.tile([64, NG], F32)
        nc.sync.dma_start(bdd[:], bdd_d[:])
        selDt = res.tile([64, NG * 32], F32R)
        nc.sync.dma_start(selDt[:], selD_d[:])
        tw_t = res.tile([RC, S], F32)
        nc.sync.dma_start(tw_t[:], tw_d[:])
        tmask_t = res.tile([RC, S], F32)
        nc.sync.dma_start(tmask_t[:], tmask_d[:])
        tg_t = res.tile([RC, S], F32)
        nc.sync.dma_start(tg_t[:], tg_d[:])
        ones_t = res.tile([RC, S], F32)
        nc.gpsimd.memset(ones_t[:], 1.0)
        zeros_t = res.tile([RC, S], F32)
        nc.gpsimd.memset(zeros_t[:], 0.0)

        # persistent PSUM accumulators
        colacc = psum.tile([96, S], F32)
        sigdens = psum.tile([64, (NG + 1) * S], F32)
        sigd = sigdens[:, 0:NG * S]
        densps = sigdens[0:RC, NG * S:(NG + 1) * S]

        # ---- main loop over quads (2 pairs each) ----
        for q in range(NQ):
            bA = sbuf.tile([128, A_W], F32R, name=f"bA_{q}", tag="bA", bufs=4)
            nc.sync.dma_start(bA[:], blobA_d[q])
            bP = sbuf.tile([4, P_W], F32R, name=f"bP_{q}", tag="bP", bufs=3)
            nc.gpsimd.dma_start(bP[:], blobP_d[q])
            bB = sbuf.tile([8, B_W], F32R, name=f"bB_{q}", tag="bB", bufs=3)
            nc.gpsimd.dma_start(bB[:], blobB_d[q])

            fs2 = None
            fsps = psum.tile([66, 2 * S], F32, name=f"fsps_{q}",
                             tag="fsps", bufs=1)
            for j in range(2):
                p_ = 2 * q + j
                c0, c1 = 2 * j, 2 * j + 1

                l1ps = psum.tile([128, 2 * S], F32, name=f"l1ps_{p_}",
                                 tag="l1ps", bufs=1)
                for i, c in enumerate((c0, c1)):
                    nc.tensor.matmul(l1ps[:, S*i:S*i+S],
                                     bP[0:4, 1024+H*c:1024+H*c+H],
                                     bP[0:4, S*c:S*c+S],
                                     start=True, stop=True)
                h1p = sbuf.tile([128, 2 * S], F32R, name=f"h1p_{p_}",
                                tag="h1p", bufs=3)
                if p_ % 2 == 0:
                    nc.scalar.activation(h1p[:], l1ps[:], AF.Relu)
                else:
                    nc.vector.tensor_scalar(h1p[:], l1ps[:], 0.0, None, ALU.max)

                l2ps = psum.tile([128, 2 * S], F32, name=f"l2ps_{p_}",
                                 tag="l2ps", bufs=2)
                h2x = []
                for i, c in enumerate((c0, c1)):
                    s_ = 4 * q + c
                    nc.tensor.matmul(l2ps[:, S*i:S*i+S],
                                     bA[:, A_W2+H*c:A_W2+H*c+H],
                                     h1p[:, S*i:S*i+S],
                                     start=True, stop=True)
                    h2 = sbuf.tile([H, S], F32R, name=f"h2_{p_}_{i}",
                                   tag=f"h2_{i}", bufs=3)
                    bias = b2g[:, s_:s_+1]
                    if s_ % 2 == 0:
                        nc.scalar.activation(h2[:], l2ps[:, S*i:S*i+S],
                                             AF.Relu, bias=bias)
                    else:
                        nc.vector.tensor_scalar(h2[:], l2ps[:, S*i:S*i+S],
                                                bias, 0.0, ALU.add, ALU.max)
                    h2x.append(h2)

                # L3 accumulate-merge: rows 0-31 featA, 32-63 featB, 64/65 sig
                for i in range(2):
                    base = A_L3 + 132*j + 66*i
                    nc.tensor.matmul(fsps[:, S*j:S*j+S],
                                     bA[:, base:base+66],
                                     h2x[i][:],
                                     start=(i == 0), stop=(i == 1))
            fs2 = sbuf.tile([66, 2 * S], F32R, name=f"fs2_{q}",
                            tag="fs2", bufs=3)
            if q % 2 == 0:
                nc.scalar.activation(fs2[:], fsps[:], AF.Copy)
            else:
                nc.vector.tensor_copy(fs2[:], fsps[:])

            for j in range(2):
                p_ = 2 * q + j
                g_, jg = divmod(p_, 32)
                fs2v = fs2[:, S*j:S*j+S]

                nc.tensor.matmul(sigd[:, S*g_:S*g_+S],
                                 bA[0:66, A_SIG+64*j:A_SIG+64*j+64],
                                 fs2v,
                                 start=(jg == 0),
                                 stop=(jg == 31 or p_ == NP - 1))
                l4c = psum.tile([128, 2 * S], F32, name=f"l4c_{p_}",
                                tag="l4c", bufs=1)
                nc.tensor.matmul(l4c[:, 0:S],
                                 bA[0:64, A_WC1+128*j:A_WC1+128*j+128],
                                 fs2[0:64, S*j:S*j+S],
                                 start=True, stop=True)
                hcx = sbuf.tile([128, S], F32R, name=f"hcx_{p_}",
                                tag="hcx", bufs=3)
                bias = bc1g[:, p_:p_+1]
                if p_ % 2 == 0:
                    nc.scalar.activation(hcx[:], l4c[:, 0:S], AF.Relu, bias=bias)
                else:
                    nc.vector.tensor_scalar(hcx[:], l4c[:, 0:S], bias, 0.0,
                                            ALU.add, ALU.max)

                nc.tensor.matmul(l4c[0:8, S:2*S],
                                 bA[:, A_WC2+8*j:A_WC2+8*j+8],
                                 hcx[:],
                                 start=True, stop=True)
                colsb = sbuf.tile([8, S], F32, name=f"colsb_{p_}",
                                  tag="colsb", bufs=3)
                nc.scalar.activation(colsb[:], l4c[0:8, S:2*S], AF.Tanh,
                                     bias=bc2g[:, p_:p_+1], scale=0.5)
                cw = sbuf.tile([8, S], F32R, name=f"cw_{p_}", tag="cw", bufs=3)
                nc.vector.scalar_tensor_tensor(
                    cw[:], colsb[:], 1.0, bB[:, 352*j+96:352*j+96+S],
                    ALU.add, ALU.mult)
                nc.tensor.matmul(colacc[:], bB[:, 352*j:352*j+96],
                                 cw[:],
                                 start=(p_ == 0), stop=(p_ == NP - 1))

        # ---- sigma tail: softplus(z) = ln(exp(min(z+bd,40)) + 1) ----
        sigs = res.tile([64, NG * S], F32)
        for g_ in range(NG):
            nc.vector.tensor_scalar(sigs[:, S*g_:S*g_+S], sigd[:, S*g_:S*g_+S],
                                    bdd[:, g_:g_+1], 40.0, ALU.add, ALU.min)
        sige = res.tile([64, NG * S], F32)
        nc.scalar.activation(sige[:], sigs[:], AF.Exp)
        sigl = res.tile([64, NG * S], F32)
        nc.scalar.activation(sigl[:], sige[:], AF.Ln, bias=1.0)
        sigv = res.tile([64, NG * S], F32R)
        nc.vector.tensor_tensor(sigv[:], sigl[:], wvd[:], ALU.mult)
        for g_ in range(NG):
            nc.tensor.matmul(densps[:], selDt[:, 32*g_:32*g_+32],
                             sigv[:, S*g_:S*g_+S],
                             start=(g_ == 0), stop=(g_ == NG - 1))

        # ---- compositing ----
        cp = res
        e_t = cp.tile([RC, S], F32)
        nc.scalar.activation(e_t[:], densps[:], AF.Exp, scale=-0.5)
        a_t = cp.tile([RC, S], F32)
        nc.vector.tensor_scalar(a_t[:], e_t[:], -1.0, 1.0, ALU.mult, ALU.add)
        Ti = cp.tile([RC, S], F32)
        nc.vector.tensor_tensor_scan(Ti[:], e_t[:], ones_t[:], 1.0,
                                     ALU.mult, ALU.mult)
        Tu = cp.tile([RC, S], F32)
        nc.gpsimd.memset(Tu[:, 0:1], 1.0)
        nc.vector.tensor_copy(Tu[:, 1:S], Ti[:, 0:S-1])
        wu = cp.tile([RC, S], F32)
        nc.vector.tensor_tensor(wu[:], Tu[:], a_t[:], ALU.mult)
        Ci = cp.tile([RC, S], F32)
        nc.vector.tensor_tensor_scan(Ci[:], wu[:], zeros_t[:], 0.0,
                                     ALU.add, ALU.add)
        Au = cp.tile([RC, S], F32)
        nc.gpsimd.memset(Au[:, 0:1], 0.0)
        nc.vector.tensor_copy(Au[:, 1:S], Ci[:, 0:S-1])
        m1 = cp.tile([RC, S], F32)
        nc.vector.tensor_scalar(m1[:], Tu[:], T_EPS, None, ALU.is_gt)
        m2 = cp.tile([RC, S], F32)
        nc.vector.tensor_scalar(m2[:], Au[:], TERM_T, None, ALU.is_le)
        wgt = cp.tile([RC, S], F32)
        nc.vector.tensor_tensor(wgt[:], wu[:], m1[:], ALU.mult)
        nc.vector.tensor_tensor(wgt[:], wgt[:], m2[:], ALU.mult)
        nc.vector.tensor_tensor(wgt[:], wgt[:], tmask_t[:], ALU.mult)
        twc = cp.tile([RC, S], F32)
        nc.vector.tensor_scalar(twc[:], tw_t[:], 1e-12, None, ALU.max)
        rcp = cp.tile([RC, S], F32)
        nc.vector.reciprocal(rcp[:], twc[:])
        rgb3 = cp.tile([RC, 3], F32)
        for ch in range(3):
            cn = cp.tile([RC, S], F32, name=f"cn_{ch}", tag="cn", bufs=2)
            nc.vector.tensor_tensor(cn[:], colacc[32*ch:32*ch+RC, :], rcp[:],
                                    ALU.mult)
            wc_ = cp.tile([RC, S], F32, name=f"wc_{ch}", tag="wc", bufs=2)
            nc.vector.tensor_tensor(wc_[:], cn[:], wgt[:], ALU.mult)
            nc.vector.tensor_reduce(rgb3[:, ch:ch+1], wc_[:],
                                    mybir.AxisListType.X, ALU.add)
        acc_t = cp.tile([RC, 1], F32)
        nc.vector.tensor_reduce(acc_t[:], wgt[:], mybir.AxisListType.X, ALU.add)
        wt_t = cp.tile([RC, S], F32)
        nc.vector.tensor_tensor(wt_t[:], wgt[:], tg_t[:], ALU.mult)
        dep_t = cp.tile([RC, 1], F32)
        nc.vector.tensor_reduce(dep_t[:], wt_t[:], mybir.AxisListType.X, ALU.add)
        nc.sync.dma_start(rgb_o[:], rgb3[:])
        nc.sync.dma_start(acc_o[:], acc_t[:])
        nc.sync.dma_start(dep_o[:], dep_t[:])

    nc.compile()
    return nc


_NC_CACHE: dict = {}


def _get_nc(NS: int) -> bass.Bass:
    if NS not in _NC_CACHE:
        _NC_CACHE[NS] = _build_nc(NS)
    return _NC_CACHE[NS]


def _host_prep(ray_origins, ray_directions, block_centers, block_radii,
               appearance_ids, exposure_values, near, far,
               W1, b1, W2, b2, Wd, bd, Wf, Wc1, bc1, Wc2, bc2, app_emb):
    f = np.float32
    o = np.asarray(ray_origins, f); d = np.asarray(ray_directions, f)
    bc_ = np.asarray(block_centers, f); br = np.asarray(block_radii, f)
    aid = np.asarray(appearance_ids).astype(np.int64)
    expo = np.asarray(exposure_values, f)
    near = np.asarray(near, f); far = np.asarray(far, f)

    oc = o[:, None, :] - bc_[None]
    a = np.sum(d * d, -1)[:, None]
    bq = f(2.0) * np.sum(oc * d[:, None, :], -1)
    cq = np.sum(oc * oc, -1) - br[None] ** 2
    disc = bq * bq - f(4.0) * a * cq
    sq = np.sqrt(np.where(disc > 0, disc, f(1.0)), dtype=f)
    sq = np.where(disc >= 0, sq, f(0.0))
    t1 = (-bq - sq) / (f(2.0) * a)
    t2 = (-bq + sq) / (f(2.0) * a)
    thit = np.where(t1 > 0, t1, t2)
    valid = (disc >= 0) & (thit > 0)
    hit = o[:, None, :] + thit[..., None] * d[:, None, :]
    dist = np.sqrt(np.sum((hit - bc_[None]) ** 2, -1), dtype=f)
    dist = np.where(valid, dist, f(np.inf))
    sel_idx = np.argsort(dist, axis=1, kind='stable')[:, :K]
    seld = np.take_along_axis(dist, sel_idx, 1)
    sel_valid = np.isfinite(seld)

    t_grid = near[:, None] + f(STEP) * np.arange(S, dtype=f)[None]
    pos = o[:, None, :] + t_grid[..., None] * d[:, None, :]
    csel = bc_[sel_idx]
    dpb = np.sqrt(np.sum((pos[:, :, None, :] - csel[:, None, :, :]) ** 2, -1),
                  dtype=f)
    inv = np.where(sel_valid[:, None, :], f(1.0) / (dpb + f(1e-6)), f(0.0))
    wsum = inv.sum(-1, keepdims=True, dtype=f)
    w = np.where(wsum > 0, inv / np.maximum(wsum, f(1e-12)), f(0.0)).astype(f)
    vis = (w >= f(VIS_T)) & sel_valid[:, None, :]
    wv = np.where(vis, w, f(0.0)).astype(f)
    tw = wv.sum(-1, dtype=f)

    W1 = np.asarray(W1, f); b1 = np.asarray(b1, f); W2 = np.asarray(W2, f)
    b2 = np.asarray(b2, f); Wd = np.asarray(Wd, f); bd = np.asarray(bd, f)
    Wf = np.asarray(Wf, f); Wc1 = np.asarray(Wc1, f); bc1 = np.asarray(bc1, f)
    Wc2 = np.asarray(Wc2, f); bc2 = np.asarray(bc2, f)
    app_emb = np.asarray(app_emb, f)

    emb_rk = app_emb[sel_idx, aid[:, None]]
    cin_const = np.concatenate([
        np.broadcast_to(d[:, None, :], (R, K, 3)),
        emb_rk,
        np.broadcast_to(expo[:, None, :], (R, K, 1)),
    ], -1).astype(f)
    Wc1b = Wc1[:, FEAT:, :]
    bc1p = bc1[sel_idx] + np.einsum('rkc,rkcd->rkd', cin_const,
                                    Wc1b[sel_idx]).astype(f)

    nr = sel_valid.sum(1)
    order = np.argsort(-nr, kind='stable')
    core_rays = [[] for _ in range(N_CORES)]
    for i, r_ in enumerate(order):
        rnd = i // N_CORES
        c_ = i % N_CORES if rnd % 2 == 0 else N_CORES - 1 - (i % N_CORES)
        core_rays[c_].append(int(r_))
    loads = [int(nr[cr].sum()) for cr in core_rays]
    NS = max(8, ((max(loads) + 7) // 8) * 8)
    NQ = NS // 4
    NP = NS // 2
    NG = (NP + 31) // 32

    in_maps = []
    for core in range(N_CORES):
        rays = core_rays[core]
        slots = []
        for rl, rg in enumerate(rays):
            for k_ in range(K):
                if sel_valid[rg, k_]:
                    slots.append((rl, rg, k_, int(sel_idx[rg, k_])))
        slots += [None] * (NS - len(slots))

        blobA = np.zeros((NQ, 128, A_W), f)
        blobP = np.zeros((NQ, 4, P_W), f)
        blobB = np.zeros((NQ, 8, B_W), f)
        b2g_a = np.zeros((H, NS), f)
        bc1g_a = np.zeros((H, NP), f)
        bc2g_a = np.zeros((8, NP), f)
        wvd_a = np.zeros((64, NG * S), f)
        bdd_a = np.zeros((64, NG), f)
        selD_a = np.zeros((64, NG * 32), f)

        for s_ in range(NS):
            sl = slots[s_]
            if sl is None:
                continue
            rl, rg, k_, b_ = sl
            q, c = divmod(s_, 4)
            j, i = divmod(c, 2)
            p_ = 2 * q + j
            g_, jg = divmod(p_, 32)
            blobP[q, 0:3, S*c:S*c+S] = pos[rg].T
            blobP[q, 3, S*c:S*c+S] = 1.0
            blobP[q, 0:3, 1024+H*c:1024+H*c+H] = W1[b_]
            blobP[q, 3, 1024+H*c:1024+H*c+H] = b1[b_]
            blobA[q, :, A_W2+H*c:A_W2+H*c+H] = W2[b_]
            base = A_L3 + 132*j + 66*i
            blobA[q, :, base+32*i:base+32*i+32] = Wf[b_]
            blobA[q, :, base+64+i] = Wd[b_, :, 0]
            blobA[q, 64+i, A_SIG+64*j + 2*jg+i] = 1.0
            blobA[q, 32*i:32*i+FEAT,
                  A_WC1+128*j+64*i:A_WC1+128*j+64*i+64] = Wc1[b_][:FEAT]
            blobA[q, 64*i:64*i+64, A_WC2+8*j+4*i:A_WC2+8*j+4*i+3] = Wc2[b_]
            b2g_a[:, s_] = b2[b_]
            bc1g_a[64*i:64*i+64, p_] = bc1p[rg, k_]
            bc2g_a[4*i:4*i+3, p_] = f(0.5) * bc2[b_]
            for ch in range(3):
                blobB[q, 4*i+ch, 352*j + 32*ch + rl] = 0.5
                blobB[q, 4*i+ch, 352*j + 96:352*j + 96 + S] = wv[rg, :, k_]
            wvd_a[2*jg+i, S*g_:S*g_+S] = wv[rg, :, k_]
            bdd_a[2*jg+i, g_] = bd[b_, 0]
            selD_a[2*jg+i, 32*g_ + rl] = 1.0

        in_maps.append(dict(
            blobA=blobA, blobP=blobP, blobB=blobB, b2g=b2g_a, bc1g=bc1g_a, bc2g=bc2g_a,
            wvd=wvd_a, bdd=bdd_a, selD=selD_a,
            tw=np.ascontiguousarray(tw[rays]),
            tmask=(t_grid[rays] < far[rays, None]).astype(f),
            tg=np.ascontiguousarray(t_grid[rays]),
        ))

    return NS, core_rays, in_maps


def kernel(**inputs):
    NS, core_rays, in_maps = _host_prep(**inputs)
    nc = _get_nc(NS)
    res = run_bass_kernel_spmd(nc, in_maps, list(range(N_CORES)))
    rgb = np.zeros((R, 3), np.float32)
    depth = np.zeros((R,), np.float32)
    acc = np.zeros((R,), np.float32)
    for core in range(N_CORES):
        r = res.results[core]
        rays = core_rays[core]
        rgb[rays] = r["rgb_o"]
        a = r["acc_o"][:, 0]
        acc[rays] = a
        depth[rays] = r["dep_o"][:, 0] / np.maximum(a, np.float32(1e-6))
    return rgb, depth, acc


# revision 25
# speedup vs baseline: 1.1210x; 1.0512x over previous
"""Trainium2 Bass kernel for nn_BlockRasterizer.

8 NeuronCores, SPMD.  Host does selection/top-K/wv + per-pair weight
gather + load balancing into fixed pair slots; all data-dependent
structure is per-core input DATA.  Device does the full per-point MLP
(fp32r matmuls), blending via selector-matmul reductions, and the
sequential compositing via DVE prefix scans.

No column tile_position is used (broken on this stack); small matmuls
are packed via row-tiling, block-diagonal K-merge of slot pairs, and
M-shifted accumulation (sigma densification into a shared PSUM bank).
"""
import sys
for p in ('/opt/trn_rl_repo', '/opt/trn_rl_repo/concourse'):
    if p not in sys.path:
        sys.path.insert(0, p)

from contextlib import ExitStack
import numpy as np

import concourse.bass as bass
import concourse.bacc as bacc
import concourse.tile as tile
from concourse import mybir
from concourse.bass_utils import run_bass_kernel_spmd

F32 = mybir.dt.float32
F32R = mybir.dt.float32r
AF = mybir.ActivationFunctionType
ALU = mybir.AluOpType

R, NB, K, S, H, FEAT, EMB, NAPP = 256, 64, 8, 256, 128, 32, 16, 100
STEP, VIS_T, TERM_T, T_EPS = 0.5, 0.01, 0.99, 1e-4
N_CORES = 8
RC = R // N_CORES

# blobP layout (per quad, [4, P_W]): slot c: pos at 256c (1024), w1 at 1024+128c
P_W = 1536
# blobA layout (per quad, [128, A_W]):
A_W2 = 0             # slot c at cols +128c (512)
A_L3 = 512           # per pair j, slot i: lhsT [128,66] (264 total)
A_SIG = 776          # per pair j: sigma densify lhsT [66,64] (128)
A_WC1 = 904          # per pair j: blockdiag [64,128] (256)
A_WC2 = 1160         # per pair j: blockdiag [128,8] (16)
A_W = 1176
# blobB layout (per quad, [8, B_W]): per pair j: selC [8,96] | cwv [8,256]
B_W = 704


def _build_nc(NS: int) -> bass.Bass:
    NQ = NS // 4
    NP = NS // 2                 # slot pairs
    NG = (NP + 31) // 32         # sigma-dense groups of 32 pairs (64 slots)
    nc = bacc.Bacc("TRN2", target_bir_lowering=False, debug=False)

    def din(name, shape, dt=F32):
        return nc.dram_tensor(name, list(shape), dt, kind="ExternalInput").ap()

    blobA_d = din("blobA", (NQ, 128, A_W), F32R)
    blobP_d = din("blobP", (NQ, 4, P_W), F32R)
    blobB_d = din("blobB", (NQ, 8, B_W), F32R)
    b2g_d = din("b2g", (H, NS))
    bc1g_d = din("bc1g", (H, NP))
    bc2g_d = din("bc2g", (8, NP))
    wvd_d = din("wvd", (64, NG * S))
    bdd_d = din("bdd", (64, NG))
    selD_d = din("selD", (64, NG * 32), F32R)
    tw_d = din("tw", (RC, S))
    tmask_d = din("tmask", (RC, S))
    tg_d = din("tg", (RC, S))

    rgb_o = nc.dram_tensor("rgb_o", [RC, 3], F32, kind="ExternalOutput").ap()
    acc_o = nc.dram_tensor("acc_o", [RC, 1], F32, kind="ExternalOutput").ap()
    dep_o = nc.dram_tensor("dep_o", [RC, 1], F32, kind="ExternalOutput").ap()

    with tile.TileContext(nc) as tc, ExitStack() as ctx:
        res = ctx.enter_context(tc.tile_pool(name="res", bufs=1))
        sbuf = ctx.enter_context(tc.tile_pool(name="sbuf", bufs=1))
        psum = ctx.enter_context(tc.tile_pool(name="psum", bufs=1, space="PSUM"))

        # ---- resident inputs ----
        b2g = res.tile([H, NS], F32)
        nc.sync.dma_start(b2g[:], b2g_d[:])
        bc1g = res.tile([H, NP], F32)
        nc.sync.dma_start(bc1g[:], bc1g_d[:])
        bc2g = res.tile([8, NP], F32)
        nc.sync.dma_start(bc2g[:], bc2g_d[:])
        wvd = res.tile([64, NG * S], F32)
        nc.sync.dma_start(wvd[:], wvd_d[:])
        bdd = res.tile([64, NG], F32)
        nc.sync.dma_start(bdd[:], bdd_d[:])
        selDt = res.tile([64, NG * 32], F32R)
        nc.sync.dma_start(selDt[:], selD_d[:])
        tw_t = res.tile([RC, S], F32)
        nc.sync.dma_start(tw_t[:], tw_d[:])
        tmask_t = res.tile([RC, S], F32)
        nc.sync.dma_start(tmask_t[:], tmask_d[:])
        tg_t = res.tile([RC, S], F32)
        nc.sync.dma_start(tg_t[:], tg_d[:])
        ones_t = res.tile([RC, S], F32)
        nc.gpsimd.memset(ones_t[:], 1.0)
        zeros_t = res.tile([RC, S], F32)
        nc.gpsimd.memset(zeros_t[:], 0.0)

        # persistent PSUM accumulators
        colacc = psum.tile([96, S], F32)
        sigdens = psum.tile([64, (NG + 1) * S], F32)
        sigd = sigdens[:, 0:NG * S]
        densps = sigdens[0:RC, NG * S:(NG + 1) * S]

        # ---- main loop over quads (2 pairs each) ----
        for q in range(NQ):
            bA = sbuf.tile([128, A_W], F32R, name=f"bA_{q}", tag="bA", bufs=4)
            nc.sync.dma_start(bA[:], blobA_d[q])
            bP = sbuf.tile([4, P_W], F32R, name=f"bP_{q}", tag="bP", bufs=3)
            nc.gpsimd.dma_start(bP[:], blobP_d[q])
            bB = sbuf.tile([8, B_W], F32R, name=f"bB_{q}", tag="bB", bufs=3)
            nc.gpsimd.dma_start(bB[:], blobB_d[q])

            fs2 = None
            fsps = psum.tile([66, 2 * S], F32, name=f"fsps_{q}",
                             tag="fsps", bufs=1)
            for j in range(2):
                p_ = 2 * q + j
                c0, c1 = 2 * j, 2 * j + 1

                l1ps = psum.tile([128, 2 * S], F32, name=f"l1ps_{p_}",
                                 tag="l1ps", bufs=1)
                for i, c in enumerate((c0, c1)):
                    nc.tensor.matmul(l1ps[:, S*i:S*i+S],
                                     bP[0:4, 1024+H*c:1024+H*c+H],
                                     bP[0:4, S*c:S*c+S],
                                     start=True, stop=True)
                h1p = sbuf.tile([128, 2 * S], F32R, name=f"h1p_{p_}",
                                tag="h1p", bufs=3)
                nc.scalar.activation(h1p[:, 0:S], l1ps[:, 0:S], AF.Relu)
                nc.vector.tensor_scalar(h1p[:, S:2*S], l1ps[:, S:2*S], 0.0,
                                        None, ALU.max)

                l2ps = psum.tile([128, 2 * S], F32, name=f"l2ps_{p_}",
                                 tag="l2ps", bufs=2)
                h2x = []
                for i, c in enumerate((c0, c1)):
                    s_ = 4 * q + c
                    nc.tensor.matmul(l2ps[:, S*i:S*i+S],
                                     bA[:, A_W2+H*c:A_W2+H*c+H],
                                     h1p[:, S*i:S*i+S],
                                     start=True, stop=True)
                    h2 = sbuf.tile([H, S], F32R, name=f"h2_{p_}_{i}",
                                   tag=f"h2_{i}", bufs=3)
                    bias = b2g[:, s_:s_+1]
                    if i == 0:
                        nc.vector.tensor_scalar(h2[:], l2ps[:, S*i:S*i+S],
                                                bias, 0.0, ALU.add, ALU.max)
                    else:
                        nc.vector.tensor_scalar(h2[:], l2ps[:, S*i:S*i+S],
                                                bias, 0.0, ALU.add, ALU.max)
                    h2x.append(h2)

                # L3 accumulate-merge: rows 0-31 featA, 32-63 featB, 64/65 sig
                for i in range(2):
                    base = A_L3 + 132*j + 66*i
                    nc.tensor.matmul(fsps[:, S*j:S*j+S],
                                     bA[:, base:base+66],
                                     h2x[i][:],
                                     start=(i == 0), stop=(i == 1))
            fs2 = sbuf.tile([66, 2 * S], F32R, name=f"fs2_{q}",
                            tag="fs2", bufs=3)
            nc.vector.tensor_copy(fs2[:], fsps[:])

            for j in range(2):
                p_ = 2 * q + j
                g_, jg = divmod(p_, 32)
                fs2v = fs2[:, S*j:S*j+S]

                nc.tensor.matmul(sigd[:, S*g_:S*g_+S],
                                 bA[0:66, A_SIG+64*j:A_SIG+64*j+64],
                                 fs2v,
                                 start=(jg == 0),
                                 stop=(jg == 31 or p_ == NP - 1))
                l4c = psum.tile([128, 2 * S], F32, name=f"l4c_{p_}",
                                tag="l4c", bufs=1)
                nc.tensor.matmul(l4c[:, 0:S],
                                 bA[0:64, A_WC1+128*j:A_WC1+128*j+128],
                                 fs2[0:64, S*j:S*j+S],
                                 start=True, stop=True)
                hcx = sbuf.tile([128, S], F32R, name=f"hcx_{p_}",
                                tag="hcx", bufs=3)
                bias = bc1g[:, p_:p_+1]
                nc.scalar.activation(hcx[:], l4c[:, 0:S], AF.Relu, bias=bias)

                nc.tensor.matmul(l4c[0:8, S:2*S],
                                 bA[:, A_WC2+8*j:A_WC2+8*j+8],
                                 hcx[:],
                                 start=True, stop=True)
                colsb = sbuf.tile([8, S], F32, name=f"colsb_{p_}",
                                  tag="colsb", bufs=3)
                nc.scalar.activation(colsb[:], l4c[0:8, S:2*S], AF.Tanh,
                                     bias=bc2g[:, p_:p_+1], scale=0.5)
                cw = sbuf.tile([8, S], F32R, name=f"cw_{p_}", tag="cw", bufs=3)
                nc.vector.scalar_tensor_tensor(
                    cw[:], colsb[:], 1.0, bB[:, 352*j+96:352*j+96+S],
                    ALU.add, ALU.mult)
                nc.tensor.matmul(colacc[:], bB[:, 352*j:352*j+96],
                                 cw[:],
                                 start=(p_ == 0), stop=(p_ == NP - 1))

        # ---- sigma tail: softplus(z) = ln(exp(min(z+bd,40)) + 1) ----
        sigs = res.tile([64, NG * S], F32)
        for g_ in range(NG):
            nc.vector.tensor_scalar(sigs[:, S*g_:S*g_+S], sigd[:, S*g_:S*g_+S],
                                    bdd[:, g_:g_+1], 40.0, ALU.add, ALU.min)
        sige = res.tile([64, NG * S], F32)
        nc.scalar.activation(sige[:], sigs[:], AF.Exp)
        sigl = res.tile([64, NG * S], F32)
        nc.scalar.activation(sigl[:], sige[:], AF.Ln, bias=1.0)
        sigv = res.tile([64, NG * S], F32R)
        nc.vector.tensor_tensor(sigv[:], sigl[:], wvd[:], ALU.mult)
        for g_ in range(NG):
            nc.tensor.matmul(densps[:], selDt[:, 32*g_:32*g_+32],
                             sigv[:, S*g_:S*g_+S],
                             start=(g_ == 0), stop=(g_ == NG - 1))

        # ---- compositing ----
        cp = res
        e_t = cp.tile([RC, S], F32)
        nc.scalar.activation(e_t[:], densps[:], AF.Exp, scale=-0.5)
        a_t = cp.tile([RC, S], F32)
        nc.vector.tensor_scalar(a_t[:], e_t[:], -1.0, 1.0, ALU.mult, ALU.add)
        Ti = cp.tile([RC, S], F32)
        nc.vector.tensor_tensor_scan(Ti[:], e_t[:], ones_t[:], 1.0,
                                     ALU.mult, ALU.mult)
        Tu = cp.tile([RC, S], F32)
        nc.gpsimd.memset(Tu[:, 0:1], 1.0)
        nc.vector.tensor_copy(Tu[:, 1:S], Ti[:, 0:S-1])
        wu = cp.tile([RC, S], F32)
        nc.vector.tensor_tensor(wu[:], Tu[:], a_t[:], ALU.mult)
        Ci = cp.tile([RC, S], F32)
        nc.vector.tensor_tensor_scan(Ci[:], wu[:], zeros_t[:], 0.0,
                                     ALU.add, ALU.add)
        Au = cp.tile([RC, S], F32)
        nc.gpsimd.memset(Au[:, 0:1], 0.0)
        nc.vector.tensor_copy(Au[:, 1:S], Ci[:, 0:S-1])
        m1 = cp.tile([RC, S], F32)
        nc.vector.tensor_scalar(m1[:], Tu[:], T_EPS, None, ALU.is_gt)
        m2 = cp.tile([RC, S], F32)
        nc.vector.tensor_scalar(m2[:], Au[:], TERM_T, None, ALU.is_le)
        wgt = cp.tile([RC, S], F32)
        nc.vector.tensor_tensor(wgt[:], wu[:], m1[:], ALU.mult)
        nc.vector.tensor_tensor(wgt[:], wgt[:], m2[:], ALU.mult)
        nc.vector.tensor_tensor(wgt[:], wgt[:], tmask_t[:], ALU.mult)
        twc = cp.tile([RC, S], F32)
        nc.vector.tensor_scalar(twc[:], tw_t[:], 1e-12, None, ALU.max)
        rcp = cp.tile([RC, S], F32)
        nc.vector.reciprocal(rcp[:], twc[:])
        rgb3 = cp.tile([RC, 3], F32)
        for ch in range(3):
            cn = cp.tile([RC, S], F32, name=f"cn_{ch}", tag="cn", bufs=2)
            nc.vector.tensor_tensor(cn[:], colacc[32*ch:32*ch+RC, :], rcp[:],
                                    ALU.mult)
            wc_ = cp.tile([RC, S], F32, name=f"wc_{ch}", tag="wc", bufs=2)
            nc.vector.tensor_tensor(wc_[:], cn[:], wgt[:], ALU.mult)
            nc.vector.tensor_reduce(rgb3[:, ch:ch+1], wc_[:],
                                    mybir.AxisListType.X, ALU.add)
        acc_t = cp.tile([RC, 1], F32)
        nc.vector.tensor_reduce(acc_t[:], wgt[:], mybir.AxisListType.X, ALU.add)
        wt_t = cp.tile([RC, S], F32)
        nc.vector.tensor_tensor(wt_t[:], wgt[:], tg_t[:], ALU.mult)
        dep_t = cp.tile([RC, 1], F32)
        nc.vector.tensor_reduce(dep_t[:], wt_t[:], mybir.AxisListType.X, ALU.add)
        nc.sync.dma_start(rgb_o[:], rgb3[:])
        nc.sync.dma_start(acc_o[:], acc_t[:])
        nc.sync.dma_start(dep_o[:], dep_t[:])

    nc.compile()
    return nc


_NC_CACHE: dict = {}


def _get_nc(NS: int) -> bass.Bass:
    if NS not in _NC_CACHE:
        _NC_CACHE[NS] = _build_nc(NS)
    return _NC_CACHE[NS]


def _host_prep(ray_origins, ray_directions, block_centers, block_radii,
               appearance_ids, exposure_values, near, far,
               W1, b1, W2, b2, Wd, bd, Wf, Wc1, bc1, Wc2, bc2, app_emb):
    f = np.float32
    o = np.asarray(ray_origins, f); d = np.asarray(ray_directions, f)
    bc_ = np.asarray(block_centers, f); br = np.asarray(block_radii, f)
    aid = np.asarray(appearance_ids).astype(np.int64)
    expo = np.asarray(exposure_values, f)
    near = np.asarray(near, f); far = np.asarray(far, f)

    oc = o[:, None, :] - bc_[None]
    a = np.sum(d * d, -1)[:, None]
    bq = f(2.0) * np.sum(oc * d[:, None, :], -1)
    cq = np.sum(oc * oc, -1) - br[None] ** 2
    disc = bq * bq - f(4.0) * a * cq
    sq = np.sqrt(np.where(disc > 0, disc, f(1.0)), dtype=f)
    sq = np.where(disc >= 0, sq, f(0.0))
    t1 = (-bq - sq) / (f(2.0) * a)
    t2 = (-bq + sq) / (f(2.0) * a)
    thit = np.where(t1 > 0, t1, t2)
    valid = (disc >= 0) & (thit > 0)
    hit = o[:, None, :] + thit[..., None] * d[:, None, :]
    dist = np.sqrt(np.sum((hit - bc_[None]) ** 2, -1), dtype=f)
    dist = np.where(valid, dist, f(np.inf))
    sel_idx = np.argsort(dist, axis=1, kind='stable')[:, :K]
    seld = np.take_along_axis(dist, sel_idx, 1)
    sel_valid = np.isfinite(seld)

    t_grid = near[:, None] + f(STEP) * np.arange(S, dtype=f)[None]
    pos = o[:, None, :] + t_grid[..., None] * d[:, None, :]
    csel = bc_[sel_idx]
    dpb = np.sqrt(np.sum((pos[:, :, None, :] - csel[:, None, :, :]) ** 2, -1),
                  dtype=f)
    inv = np.where(sel_valid[:, None, :], f(1.0) / (dpb + f(1e-6)), f(0.0))
    wsum = inv.sum(-1, keepdims=True, dtype=f)
    w = np.where(wsum > 0, inv / np.maximum(wsum, f(1e-12)), f(0.0)).astype(f)
    vis = (w >= f(VIS_T)) & sel_valid[:, None, :]
    wv = np.where(vis, w, f(0.0)).astype(f)
    tw = wv.sum(-1, dtype=f)

    W1 = np.asarray(W1, f); b1 = np.asarray(b1, f); W2 = np.asarray(W2, f)
    b2 = np.asarray(b2, f); Wd = np.asarray(Wd, f); bd = np.asarray(bd, f)
    Wf = np.asarray(Wf, f); Wc1 = np.asarray(Wc1, f); bc1 = np.asarray(bc1, f)
    Wc2 = np.asarray(Wc2, f); bc2 = np.asarray(bc2, f)
    app_emb = np.asarray(app_emb, f)

    emb_rk = app_emb[sel_idx, aid[:, None]]
    cin_const = np.concatenate([
        np.broadcast_to(d[:, None, :], (R, K, 3)),
        emb_rk,
        np.broadcast_to(expo[:, None, :], (R, K, 1)),
    ], -1).astype(f)
    Wc1b = Wc1[:, FEAT:, :]
    bc1p = bc1[sel_idx] + np.einsum('rkc,rkcd->rkd', cin_const,
                                    Wc1b[sel_idx]).astype(f)

    nr = sel_valid.sum(1)
    order = np.argsort(-nr, kind='stable')
    core_rays = [[] for _ in range(N_CORES)]
    for i, r_ in enumerate(order):
        rnd = i // N_CORES
        c_ = i % N_CORES if rnd % 2 == 0 else N_CORES - 1 - (i % N_CORES)
        core_rays[c_].append(int(r_))
    loads = [int(nr[cr].sum()) for cr in core_rays]
    NS = max(8, ((max(loads) + 7) // 8) * 8)
    NQ = NS // 4
    NP = NS // 2
    NG = (NP + 31) // 32

    in_maps = []
    for core in range(N_CORES):
        rays = core_rays[core]
        slots = []
        for rl, rg in enumerate(rays):
            for k_ in range(K):
                if sel_valid[rg, k_]:
                    slots.append((rl, rg, k_, int(sel_idx[rg, k_])))
        slots += [None] * (NS - len(slots))

        blobA = np.zeros((NQ, 128, A_W), f)
        blobP = np.zeros((NQ, 4, P_W), f)
        blobB = np.zeros((NQ, 8, B_W), f)
        b2g_a = np.zeros((H, NS), f)
        bc1g_a = np.zeros((H, NP), f)
        bc2g_a = np.zeros((8, NP), f)
        wvd_a = np.zeros((64, NG * S), f)
        bdd_a = np.zeros((64, NG), f)
        selD_a = np.zeros((64, NG * 32), f)

        for s_ in range(NS):
            sl = slots[s_]
            if sl is None:
                continue
            rl, rg, k_, b_ = sl
            q, c = divmod(s_, 4)
            j, i = divmod(c, 2)
            p_ = 2 * q + j
            g_, jg = divmod(p_, 32)
            blobP[q, 0:3, S*c:S*c+S] = pos[rg].T
            blobP[q, 3, S*c:S*c+S] = 1.0
            blobP[q, 0:3, 1024+H*c:1024+H*c+H] = W1[b_]
            blobP[q, 3, 1024+H*c:1024+H*c+H] = b1[b_]
            blobA[q, :, A_W2+H*c:A_W2+H*c+H] = W2[b_]
            base = A_L3 + 132*j + 66*i
            blobA[q, :, base+32*i:base+32*i+32] = Wf[b_]
            blobA[q, :, base+64+i] = Wd[b_, :, 0]
            blobA[q, 64+i, A_SIG+64*j + 2*jg+i] = 1.0
            blobA[q, 32*i:32*i+FEAT,
                  A_WC1+128*j+64*i:A_WC1+128*j+64*i+64] = Wc1[b_][:FEAT]
            blobA[q, 64*i:64*i+64, A_WC2+8*j+4*i:A_WC2+8*j+4*i+3] = Wc2[b_]
            b2g_a[:, s_] = b2[b_]
            bc1g_a[64*i:64*i+64, p_] = bc1p[rg, k_]
            bc2g_a[4*i:4*i+3, p_] = f(0.5) * bc2[b_]
            for ch in range(3):
                blobB[q, 4*i+ch, 352*j + 32*ch + rl] = 0.5
                blobB[q, 4*i+ch, 352*j + 96:352*j + 96 + S] = wv[rg, :, k_]
            wvd_a[2*jg+i, S*g_:S*g_+S] = wv[rg, :, k_]
            bdd_a[2*jg+i, g_] = bd[b_, 0]
            selD_a[2*jg+i, 32*g_ + rl] = 1.0

        in_maps.append(dict(
            blobA=blobA, blobP=blobP, blobB=blobB, b2g=b2g_a, bc1g=bc1g_a, bc2g=bc2g_a,
            wvd=wvd_a, bdd=bdd_a, selD=selD_a,
            tw=np.ascontiguousarray(tw[rays]),
            tmask=(t_grid[rays] < far[rays, None]).astype(f),
            tg=np.ascontiguousarray(t_grid[rays]),
        ))

    return NS, core_rays, in_maps


def kernel(**inputs):
    NS, core_rays, in_maps = _host_prep(**inputs)
    nc = _get_nc(NS)
    res = run_bass_kernel_spmd(nc, in_maps, list(range(N_CORES)))
    rgb = np.zeros((R, 3), np.float32)
    depth = np.zeros((R,), np.float32)
    acc = np.zeros((R,), np.float32)
    for core in range(N_CORES):
        r = res.results[core]
        rays = core_rays[core]
        rgb[rays] = r["rgb_o"]
        a = r["acc_o"][:, 0]
        acc[rays] = a
        depth[rays] = r["dep_o"][:, 0] / np.maximum(a, np.float32(1e-6))
    return rgb, depth, acc
